# revision 17
# baseline (speedup 1.0000x reference)
"""GQA attention kernel for 8 Trainium2 NeuronCores.

Problem: B=2, S=2048, D=1024, 16 Q heads / 4 KV heads (GQA), causal,
y = softmax((x@wq+bq)(x@wk+bk)^T / 8, causal) @ (x@wv+bv) @ wo + bo

Sharding: core c -> (batch b = c//4, kv-group g = c%4). Each core computes
its batch's attention for 4 Q heads (= 1 KV head) and the partial output
projection through wo[g*256:(g+1)*256, :]. Host sums the 4 partials per
batch and adds bo.

Per-core kernel (matmul operands bf16, accumulation fp32 in PSUM):
  xT is pre-transposed on host -> DMA'd straight into SBUF [D, S] tiles.
  qp[mc] = (wq^T xT)*0.125 + bq  head-pairs stacked [128, S] (rows 0-63 =
     head 2mc, 64-127 = head 2mc+1)
  kT2   = [k; k] duplicated [128, S]; v transposed into vA [kpos, kt, 65]
     with a ones column (row 64 of AV output = softmax denominator)
  attention per (qb, head-pair): per key tile kt two score matmuls run
     CONCURRENTLY on the PE via row tiling (rows 0-63 / 64-127 of the
     array, auto tile_position from base_partition) into adjacent PSUM
     banks [128, 2, 512]; one Exp over both banks; causal masks; two AV
     matmuls accumulate [65, 512] per head.  Normalize: ACT reciprocal
     of row 64, gpsimd partition_broadcast, vector multiply (+bv).
  out-proj interleaved per qb: y_partial tiles DMA'd to DRAM fp32.
"""

import os
import sys
from contextlib import ExitStack

import numpy as np
import ml_dtypes

if "/opt/trn_rl_repo" not in sys.path:
    sys.path.insert(0, "/opt/trn_rl_repo")

import concourse.bass as bass
import concourse.tile as tile
from concourse import bacc, mybir
from concourse.masks import make_identity

B, S, D = 2, 2048, 1024
H, KVH, HD = 16, 4, 64
GQ = H // KVH        # 4 q heads per core
DG = GQ * HD         # 256 q dims per core
P = 128
KC = D // P          # 8 contraction chunks over D
NKT = S // P         # 16 key tiles
NQB = S // 512       # 4 query blocks
N_CORES = 8

DT = mybir.dt.float32
DTB = mybir.dt.bfloat16
AF = mybir.ActivationFunctionType
BF16 = ml_dtypes.bfloat16

_CACHE = {}


def build_nc():
    nc = bacc.Bacc(
        "TRN2",
        target_bir_lowering=False,
        debug=False,
        enable_asserts=False,
        num_devices=N_CORES,
    )
    xtd = nc.dram_tensor("xtd", [D, S], DTB, kind="ExternalInput").ap()
    wqd = nc.dram_tensor("wqd", [D, DG], DTB, kind="ExternalInput").ap()
    wkd = nc.dram_tensor("wkd", [D, HD], DTB, kind="ExternalInput").ap()
    wvd = nc.dram_tensor("wvd", [D, HD], DTB, kind="ExternalInput").ap()
    wod = nc.dram_tensor("wod", [DG, D], DTB, kind="ExternalInput").ap()
    bqd = nc.dram_tensor("bqd", [DG, 1], DT, kind="ExternalInput").ap()
    bkd = nc.dram_tensor("bkd", [HD, 1], DT, kind="ExternalInput").ap()
    bvd = nc.dram_tensor("bvd", [DG, 1], DT, kind="ExternalInput").ap()
    out_p = nc.dram_tensor("out_p", [S, D], DT, kind="ExternalOutput").ap()

    with tile.TileContext(nc) as tc, ExitStack() as ctx:
        consts = ctx.enter_context(tc.tile_pool(name="consts", bufs=1))
        vtmp = ctx.enter_context(tc.tile_pool(name="vtmp", bufs=2))
        etp = ctx.enter_context(tc.tile_pool(name="etp", bufs=3))
        ysb = ctx.enter_context(tc.tile_pool(name="ysb", bufs=3))
        recp = ctx.enter_context(tc.tile_pool(name="recp", bufs=2))
        rbp = ctx.enter_context(tc.tile_pool(name="rbp", bufs=2))
        accsb = ctx.enter_context(tc.tile_pool(name="accsb", bufs=3))
        # PSUM budget (16KB/partition = 8 banks):
        #   psS  "sd"   [P,2,512]f32 x2 bufs = 4 banks
        #   psAcc acc0/acc1 [65,512]f32 x1 buf each = 2 banks
        #   psOut "ps"  [P,512]f32 x2 bufs = 2 banks (proj, v-transp, out-proj)
        psS = ctx.enter_context(tc.tile_pool(name="psS", bufs=2, space="PSUM"))
        psAcc = ctx.enter_context(tc.tile_pool(name="psAcc", bufs=1, space="PSUM"))
        psOut = ctx.enter_context(tc.tile_pool(name="psOut", bufs=2, space="PSUM"))

        ident = consts.tile([P, P], DTB, tag="ident")
        make_identity(nc, ident)
        # Causal 0/1 masks for the four diagonal positions. The affine base
        # qb*512 - kt*128 = -128*di depends only on di = kt - 4*qb, so all
        # four masks are fixed tiles ([128, 2, 512]: both head-parity slices
        # carry the same mask).
        dmask = []
        for di in range(4):
            mt = consts.tile([P, 2, 512], DTB, tag=f"dmask{di}", name=f"dmask{di}")
            nc.gpsimd.memset(mt, 1.0)
            for hp in range(2):
                nc.gpsimd.affine_select(
                    out=mt[:, hp], in_=mt[:, hp], pattern=[[1, 512]],
                    compare_op=mybir.AluOpType.is_ge, fill=0.0,
                    base=-128 * di, channel_multiplier=-1)
            dmask.append(mt)

        xT = [consts.tile([P, S], DTB, tag=f"xT{dc}", name=f"xT{dc}") for dc in range(KC)]
        qp = [consts.tile([P, S], DTB, tag=f"qp{mc}", name=f"qp{mc}") for mc in range(2)]
        kT2 = consts.tile([P, S], DTB, tag="kT2")
        vA = consts.tile([P, NKT, HD + 1], DTB, tag="vA")
        oT = [consts.tile([P, S], DTB, tag=f"oT{c}", name=f"oT{c}") for c in range(2)]

        wq_sb = [consts.tile([P, DG], DTB, tag=f"wq{kc}", name=f"wq{kc}") for kc in range(KC)]
        wkv_sb = [consts.tile([P, 2 * HD], DTB, tag=f"wkv{kc}", name=f"wkv{kc}") for kc in range(KC)]
        wo_sb = [consts.tile([P, D], DTB, tag=f"wo{c}", name=f"wo{c}") for c in range(2)]
        bq_sb = [consts.tile([P, 1], DT, tag=f"bq{mc}", name=f"bq{mc}") for mc in range(2)]
        bk_sb = consts.tile([HD, 1], DT, tag="bk")
        bv_sb = [consts.tile([P, 1], DT, tag=f"bv{c}", name=f"bv{c}") for c in range(2)]

        # ---- weight / bias / xT loads (round-robin over HWDGE queues) ----
        QS = [nc.sync, nc.scalar, nc.gpsimd]
        qi = 0

        def dq():
            nonlocal qi
            qi += 1
            return QS[qi % len(QS)]

        # kv weights + first x column-chunk first so the PE can start ASAP;
        # x is loaded in per-nb column chunks to pipeline DMA with proj.
        for kc in range(KC):
            dq().dma_start(wkv_sb[kc][:, 0:HD], wkd[kc * P:(kc + 1) * P, :])
            dq().dma_start(wkv_sb[kc][:, HD:2 * HD], wvd[kc * P:(kc + 1) * P, :])
        dq().dma_start(bk_sb, bkd[:, :])
        for dc in range(KC):
            QS[dc % len(QS)].dma_start(xT[dc][:, 0:512], xtd[dc * P:(dc + 1) * P, 0:512])
        for kc in range(KC):
            dq().dma_start(wq_sb[kc], wqd[kc * P:(kc + 1) * P, :])
        for c in range(2):
            dq().dma_start(bq_sb[c], bqd[c * P:(c + 1) * P, :])
            dq().dma_start(bv_sb[c], bvd[c * P:(c + 1) * P, :])
        for nb in range(1, 4):
            for dc in range(KC):
                QS[dc % len(QS)].dma_start(
                    xT[dc][:, nb * 512:(nb + 1) * 512],
                    xtd[dc * P:(dc + 1) * P, nb * 512:(nb + 1) * 512])
        for c in range(2):
            dq().dma_start(wo_sb[c], wod[c * P:(c + 1) * P, :])
        nc.vector.memset(vA[:, :, HD:HD + 1], 1.0)

        # ---- phase 1: projections (v transposed after q-proj so the PE
        # isn't gated on the vt copy) ----
        vts = []
        for nb in range(4):
            sl = slice(nb * 512, (nb + 1) * 512)
            ps2 = psOut.tile([P, 512], DT, tag="ps")
            for kc in range(KC):
                nc.tensor.matmul(
                    ps2, wkv_sb[kc], xT[kc][:, sl],
                    start=(kc == 0), stop=(kc == KC - 1))
            nc.scalar.activation(kT2[0:HD, sl], ps2[0:HD, :], AF.Identity, bias=bk_sb)
            nc.vector.tensor_copy(kT2[HD:2 * HD, sl], kT2[0:HD, sl])
            vt = vtmp.tile([HD, 512], DTB, tag=f"vtmp{nb}", name=f"vt{nb}")
            nc.vector.tensor_copy(vt, ps2[HD:2 * HD, :])
            vts.append(vt)
            for mc in range(2):
                ps = psOut.tile([P, 512], DT, tag="ps")
                for kc in range(KC):
                    nc.tensor.matmul(
                        ps, wq_sb[kc][:, mc * P:(mc + 1) * P], xT[kc][:, sl],
                        start=(kc == 0), stop=(kc == KC - 1))
                nc.scalar.activation(
                    qp[mc][:, sl], ps, AF.Identity, bias=bq_sb[mc], scale=0.125)

        def v_transposes(nb):
            for j in range(4):
                kt = nb * 4 + j
                vps = psOut.tile([P, HD], DTB, tag="ps", name="vps")
                nc.tensor.transpose(
                    vps, vts[nb][:, j * P:(j + 1) * P], ident[0:HD, 0:HD])
                nc.vector.tensor_copy(vA[:, kt, 0:HD], vps)

        # ---- phase 2: attention + interleaved output projection ----
        # v-transposes for block nb are emitted just before the attention
        # block that first needs them, so they don't head-of-line block the
        # PE on the vt copy.
        ydma = 0
        for qb in range(NQB):
            v_transposes(qb)
            qsl = slice(qb * 512, (qb + 1) * 512)
            nkt = 4 * (qb + 1)
            for mc in range(2):
                acc = [psAcc.tile([HD + 1, 512], DT, tag=f"acc{hh}",
                                  name=f"acc{hh}") for hh in range(2)]
                for kt in range(nkt):
                    ksl = slice(kt * P, (kt + 1) * P)
                    sps = psS.tile([P, 2, 512], DT, tag="sd")
                    nc.tensor.matmul(
                        sps[:, 0], kT2[0:HD, ksl], qp[mc][0:HD, qsl],
                        start=True, stop=True)
                    nc.tensor.matmul(
                        sps[:, 1], kT2[HD:2 * HD, ksl], qp[mc][HD:2 * HD, qsl],
                        start=True, stop=True)
                    et = etp.tile([P, 2, 512], DTB, tag="et")
                    nc.scalar.activation(et, sps, AF.Exp)
                    if kt >= 4 * qb:
                        di = kt - 4 * qb
                        meng = nc.vector if (kt % 2 == 0) else nc.gpsimd
                        meng.tensor_mul(et, et, dmask[di])
                    for hh in range(2):
                        nc.tensor.matmul(
                            acc[hh], vA[:, kt, :], et[:, hh],
                            start=(kt == 0), stop=(kt == nkt - 1))
                for hh in range(2):
                    # Drain PSUM to SBUF immediately so the acc bank frees
                    # for the next (qb, mc) stream; normalize from SBUF.
                    den = recp.tile([1, 512], DT, tag="den")
                    nc.vector.tensor_copy(den, acc[hh][HD:HD + 1, :])
                    sba = accsb.tile([HD, 512], DT, tag="sba")
                    nc.vector.tensor_copy(sba, acc[hh][0:HD, :])
                    rec = recp.tile([1, 512], DT, tag="rec")
                    nc.vector.reciprocal_approx_fast(rec, den)
                    rbs = rbp.tile([HD, 512], DT, tag="rbs")
                    nc.gpsimd.partition_broadcast(rbs, rec)
                    r0 = hh * HD
                    nc.vector.tensor_mul(
                        oT[mc][r0:r0 + HD, qsl], sba, rbs)
                    nc.vector.tensor_scalar_add(
                        oT[mc][r0:r0 + HD, qsl], oT[mc][r0:r0 + HD, qsl],
                        bv_sb[mc][r0:r0 + HD, :])
            # output projection for this query block (all 4 heads ready)
            for st in range(qb * 4, qb * 4 + 4):
                for nb2 in range(2):
                    yps = psOut.tile([P, 512], DT, tag="ps")
                    for c in range(2):
                        nc.tensor.matmul(
                            yps, oT[c][:, st * P:(st + 1) * P],
                            wo_sb[c][:, nb2 * 512:(nb2 + 1) * 512],
                            start=(c == 0), stop=(c == 1))
                    yt = ysb.tile([P, 512], DT, tag="y")
                    if ydma % 2 == 0:
                        nc.vector.tensor_copy(yt, yps)
                    else:
                        nc.scalar.activation(yt, yps, AF.Identity)
                    QS[ydma % len(QS)].dma_start(
                        out_p[st * P:(st + 1) * P, nb2 * 512:(nb2 + 1) * 512], yt)
                    ydma += 1

    nc.compile()
    return nc


def kernel(x, mask, wq, bq, wk, bk, wv, bv, wo, bo):
    x = np.asarray(x, dtype=np.float32)
    wq = np.asarray(wq, dtype=np.float32)
    wk = np.asarray(wk, dtype=np.float32)
    wv = np.asarray(wv, dtype=np.float32)
    wo = np.asarray(wo, dtype=np.float32)
    bq = np.asarray(bq, dtype=np.float32)
    bk = np.asarray(bk, dtype=np.float32)
    bv = np.asarray(bv, dtype=np.float32)
    bo = np.asarray(bo, dtype=np.float32)

    wqb = wq.astype(BF16)
    wkb = wk.astype(BF16)
    wvb = wv.astype(BF16)
    wob = wo.astype(BF16)
    xtb = np.ascontiguousarray(x.transpose(0, 2, 1)).astype(BF16)  # [B, D, S]

    in_maps = []
    for c in range(N_CORES):
        b, g = c // 4, c % 4
        sq = slice(g * DG, (g + 1) * DG)
        sk = slice(g * HD, (g + 1) * HD)
        in_maps.append({
            "xtd": xtb[b],
            "wqd": np.ascontiguousarray(wqb[:, sq]),
            "wkd": np.ascontiguousarray(wkb[:, sk]),
            "wvd": np.ascontiguousarray(wvb[:, sk]),
            "wod": np.ascontiguousarray(wob[sq, :]),
            "bqd": np.ascontiguousarray((bq[sq] * 0.125).reshape(DG, 1)),
            "bkd": np.ascontiguousarray(bk[sk].reshape(HD, 1)),
            "bvd": np.ascontiguousarray(np.tile(bv[sk], GQ).reshape(DG, 1)),
        })

    results = _run(in_maps)

    out = np.empty((B, S, D), dtype=np.float32)
    for b in range(B):
        acc = results[b * 4 + 0]["out_p"].astype(np.float64)
        for g in range(1, 4):
            acc += results[b * 4 + g]["out_p"]
        out[b] = (acc + bo).astype(np.float32)
    return out


def _get_runner():
    """Build (once) a jitted shard_map callable executing the compiled
    kernel on 8 cores. Adapted from concourse.bass2jax.run_bass_via_pjrt,
    minus output-buffer donation so the callable is re-invokable for
    timing."""
    if "runner" in _CACHE:
        return _CACHE["runner"]
    import jax
    from jax.experimental.shard_map import shard_map
    from jax.sharding import Mesh, PartitionSpec
    from concourse import bass2jax
    from concourse.bass2jax import _bass_exec_p, install_neuronx_cc_hook

    install_neuronx_cc_hook()
    nc = build_nc()
    partition_name = (
        nc.partition_id_tensor.name if nc.partition_id_tensor else None
    )

    in_names, out_names, out_avals, zero_outs = [], [], [], []
    for alloc in nc.m.functions[0].allocations:
        if not isinstance(alloc, mybir.MemoryLocationSet):
            continue
        name = alloc.memorylocations[0].name
        if alloc.kind == "ExternalInput":
            if name != partition_name:
                in_names.append(name)
        elif alloc.kind == "ExternalOutput":
            out_names.append(name)
            shape = tuple(alloc.tensor_shape)
            dtype = mybir.dt.np(alloc.dtype)
            out_avals.append(jax.core.ShapedArray(shape, dtype))
            zero_outs.append(np.zeros(shape, dtype))
    n_params = len(in_names)
    all_names = in_names + out_names
    if partition_name is not None:
        all_names = all_names + [partition_name]

    def _body(*args):
        operands = list(args)
        if partition_name is not None:
            operands.append(bass2jax.partition_id_tensor())
        outs = _bass_exec_p.bind(
            *operands,
            out_avals=tuple(out_avals),
            in_names=tuple(all_names),
            out_names=tuple(out_names),
            lowering_input_output_aliases=(),
            sim_require_finite=True,
            sim_require_nnan=True,
            nc=nc,
        )
        return tuple(outs)

    devices = jax.devices()[:N_CORES]
    mesh = Mesh(np.asarray(devices), ("core",))
    n_all = n_params + len(out_names)
    sharded = jax.jit(
        shard_map(
            _body,
            mesh=mesh,
            in_specs=(PartitionSpec("core"),) * n_all,
            out_specs=(PartitionSpec("core"),) * len(out_names),
            check_rep=False,
        ),
        keep_unused=True,
    )
    runner = {
        "sharded": sharded,
        "in_names": in_names,
        "out_names": out_names,
        "out_avals": out_avals,
        "zero_outs": zero_outs,
        "mesh": mesh,
        "nc": nc,
    }
    _CACHE["runner"] = runner
    return runner


def _run(in_maps):
    r = _get_runner()
    concat_in = [
        np.concatenate([np.asarray(in_maps[c][n]) for c in range(N_CORES)], axis=0)
        for n in r["in_names"]
    ]
    concat_zeros = [
        np.zeros((N_CORES * z.shape[0], *z.shape[1:]), z.dtype)
        for z in r["zero_outs"]
    ]
    out_arrs = r["sharded"](*concat_in, *concat_zeros)
    _CACHE["last_args"] = (concat_in, concat_zeros)
    return [
        {
            n: np.asarray(out_arrs[i]).reshape(
                N_CORES, *r["out_avals"][i].shape
            )[c]
            for i, n in enumerate(r["out_names"])
        }
        for c in range(N_CORES)
    ]


def bench(iters=10):
    """Re-execute the last-run kernel with device-resident inputs and
    return per-call wall times (s). Outputs stay on device."""
    import time as _time
    import jax
    from jax.sharding import NamedSharding, PartitionSpec

    r = _CACHE["runner"]
    concat_in, concat_zeros = _CACHE["last_args"]
    sh = NamedSharding(r["mesh"], PartitionSpec("core"))
    dev_args = [jax.device_put(a, sh) for a in (*concat_in, *concat_zeros)]
    for a in dev_args:
        a.block_until_ready()
    times = []
    for _ in range(iters):
        t0 = _time.perf_counter()
        outs = r["sharded"](*dev_args)
        for o in outs:
            o.block_until_ready()
        times.append(_time.perf_counter() - t0)
    return times


def profile_exec_ns(outdir="/tmp/kernel_ntff"):
    """Capture an NTFF profile of one execution on all 8 cores and return
    (max_core_span_ns, per_core_span_ns). The span is the on-device NEFF
    execution time: last instruction end - first instruction start."""
    import ctypes
    import glob
    import json
    import shutil
    import jax
    from jax.sharding import NamedSharding, PartitionSpec

    r = _CACHE["runner"]
    concat_in, concat_zeros = _CACHE["last_args"]
    sh = NamedSharding(r["mesh"], PartitionSpec("core"))
    dev_args = [jax.device_put(a, sh) for a in (*concat_in, *concat_zeros)]
    for a in dev_args:
        a.block_until_ready()
    outs = r["sharded"](*dev_args)  # warm
    for o in outs:
        o.block_until_ready()

    lib = ctypes.CDLL("/opt/axon/libaxon_pjrt.so")
    if not hasattr(lib, "axon_start_nrt_profile"):
        return None, None
    lib.axon_start_nrt_profile.argtypes = [
        ctypes.POINTER(ctypes.c_int64), ctypes.c_size_t]
    lib.axon_start_nrt_profile.restype = ctypes.c_int64
    lib.axon_stop_nrt_profile.argtypes = [ctypes.c_char_p]
    lib.axon_stop_nrt_profile.restype = ctypes.c_int64

    shutil.rmtree(outdir, ignore_errors=True)
    os.makedirs(outdir, exist_ok=True)
    ids = (ctypes.c_int64 * N_CORES)(*range(N_CORES))
    rc = lib.axon_start_nrt_profile(ids, N_CORES)
    if rc != 0:
        return None, None
    outs = r["sharded"](*dev_args)
    for o in outs:
        o.block_until_ready()
    n = lib.axon_stop_nrt_profile(str(outdir).encode())
    if n <= 0:
        return None, None

    import gauge.profiler
    from concourse._compat import FishPath

    profile = gauge.profiler.Profile(
        profile_path=FishPath(outdir),
        kernel_dev_mode=True,
        profile_on_exit=False,
        bass_kernel=_CACHE["runner"]["nc"].m,
        offline_processing=True,
        fname="*_body*",
    )
    profile.convert_ntffs_to_json(tuple(range(N_CORES)))
    spans = {}
    for c in range(N_CORES):
        jp = os.path.join(outdir, f"ntff_{c}.json")
        if not os.path.exists(jp):
            continue
        d = json.load(open(jp))
        insts = d.get("instruction", [])
        if not insts:
            continue
        t0 = min(i["timestamp"] for i in insts)
        t1 = max(i["timestamp"] + i["duration"] for i in insts)
        spans[c] = t1 - t0
    if not spans:
        return None, None
    return max(spans.values()), spans


# revision 19
# speedup vs baseline: 1.4759x; 1.4759x over previous
"""GQA attention kernel for 8 Trainium2 NeuronCores.

Problem: B=2, S=2048, D=1024, 16 Q heads / 4 KV heads (GQA), causal,
y = softmax((x@wq+bq)(x@wk+bk)^T / 8, causal) @ (x@wv+bv) @ wo + bo

Sharding: core c -> (batch b = c//4, kv-group g = c%4). Each core computes
its batch's attention for 4 Q heads (= 1 KV head) and the partial output
projection through wo[g*256:(g+1)*256, :]. Host sums the 4 partials per
batch and adds bo.

Per-core kernel (matmul operands bf16, accumulation fp32 in PSUM):
  xT is pre-transposed on host -> DMA'd straight into SBUF [D, S] tiles.
  qp[mc] = (wq^T xT)*0.125 + bq  head-pairs stacked [128, S] (rows 0-63 =
     head 2mc, 64-127 = head 2mc+1)
  kT2   = [k; k] duplicated [128, S]; v transposed into vA [kpos, kt, 65]
     with a ones column (row 64 of AV output = softmax denominator)
  attention per (qb, head-pair): per key tile kt two score matmuls run
     CONCURRENTLY on the PE via row tiling (rows 0-63 / 64-127 of the
     array, auto tile_position from base_partition) into adjacent PSUM
     banks [128, 2, 512]; one Exp over both banks; causal masks; two AV
     matmuls accumulate [65, 512] per head.  Normalize: ACT reciprocal
     of row 64, gpsimd partition_broadcast, vector multiply (+bv).
  out-proj interleaved per qb: y_partial tiles DMA'd to DRAM fp32.
"""

import os
import sys
from contextlib import ExitStack

import numpy as np
import ml_dtypes

if "/opt/trn_rl_repo" not in sys.path:
    sys.path.insert(0, "/opt/trn_rl_repo")

import concourse.bass as bass
import concourse.tile as tile
from concourse import bacc, mybir
from concourse.masks import make_identity

B, S, D = 2, 2048, 1024
H, KVH, HD = 16, 4, 64
GQ = H // KVH        # 4 q heads per core
DG = GQ * HD         # 256 q dims per core
P = 128
KC = D // P          # 8 contraction chunks over D
NKT = S // P         # 16 key tiles
NQB = S // 512       # 4 query blocks
N_CORES = 8

DT = mybir.dt.float32
DTB = mybir.dt.bfloat16
AF = mybir.ActivationFunctionType
BF16 = ml_dtypes.bfloat16

_CACHE = {}


def build_nc():
    nc = bacc.Bacc(
        "TRN2",
        target_bir_lowering=False,
        debug=False,
        enable_asserts=False,
        num_devices=N_CORES,
    )
    xtd = nc.dram_tensor("xtd", [D, S], DTB, kind="ExternalInput").ap()
    wqd = nc.dram_tensor("wqd", [D, DG], DTB, kind="ExternalInput").ap()
    wkd = nc.dram_tensor("wkd", [D, HD], DTB, kind="ExternalInput").ap()
    wvd = nc.dram_tensor("wvd", [D, HD], DTB, kind="ExternalInput").ap()
    wod = nc.dram_tensor("wod", [DG, D], DTB, kind="ExternalInput").ap()
    bqd = nc.dram_tensor("bqd", [DG, 1], DT, kind="ExternalInput").ap()
    bkd = nc.dram_tensor("bkd", [HD, 1], DT, kind="ExternalInput").ap()
    bvd = nc.dram_tensor("bvd", [DG, 1], DT, kind="ExternalInput").ap()
    out_p = nc.dram_tensor("out_p", [S, D], DT, kind="ExternalOutput").ap()

    with tile.TileContext(nc) as tc, ExitStack() as ctx:
        consts = ctx.enter_context(tc.tile_pool(name="consts", bufs=1))
        vtmp = ctx.enter_context(tc.tile_pool(name="vtmp", bufs=2))
        etp = ctx.enter_context(tc.tile_pool(name="etp", bufs=3))
        ysb = ctx.enter_context(tc.tile_pool(name="ysb", bufs=3))
        recp = ctx.enter_context(tc.tile_pool(name="recp", bufs=2))
        rbp = ctx.enter_context(tc.tile_pool(name="rbp", bufs=2))
        accsb = ctx.enter_context(tc.tile_pool(name="accsb", bufs=3))
        # PSUM budget (16KB/partition = 8 banks):
        #   psS  "sd"   [P,2,512]f32 x2 bufs = 4 banks
        #   psAcc acc0/acc1 [65,512]f32 x1 buf each = 2 banks
        #   psOut "ps"  [P,512]f32 x2 bufs = 2 banks (proj, v-transp, out-proj)
        psS = ctx.enter_context(tc.tile_pool(name="psS", bufs=2, space="PSUM"))
        psAcc = ctx.enter_context(tc.tile_pool(name="psAcc", bufs=1, space="PSUM"))
        psOut = ctx.enter_context(tc.tile_pool(name="psOut", bufs=2, space="PSUM"))

        ident = consts.tile([P, P], DTB, tag="ident")
        make_identity(nc, ident)
        # Causal 0/1 masks for the four diagonal positions. The affine base
        # qb*512 - kt*128 = -128*di depends only on di = kt - 4*qb, so all
        # four masks are fixed tiles ([128, 2, 512]: both head-parity slices
        # carry the same mask).
        dmask = []
        for di in range(4):
            mt = consts.tile([P, 2, 512], DTB, tag=f"dmask{di}", name=f"dmask{di}")
            nc.gpsimd.memset(mt, 1.0)
            for hp in range(2):
                nc.gpsimd.affine_select(
                    out=mt[:, hp], in_=mt[:, hp], pattern=[[1, 512]],
                    compare_op=mybir.AluOpType.is_ge, fill=0.0,
                    base=-128 * di, channel_multiplier=-1)
            dmask.append(mt)

        xT = [consts.tile([P, S], DTB, tag=f"xT{dc}", name=f"xT{dc}") for dc in range(KC)]
        qp = [consts.tile([P, S], DTB, tag=f"qp{mc}", name=f"qp{mc}") for mc in range(2)]
        kT2 = consts.tile([P, S], DTB, tag="kT2")
        vA = consts.tile([P, NKT, HD + 1], DTB, tag="vA")
        oT = [consts.tile([P, S], DTB, tag=f"oT{c}", name=f"oT{c}") for c in range(2)]

        wq_sb = [consts.tile([P, DG], DTB, tag=f"wq{kc}", name=f"wq{kc}") for kc in range(KC)]
        wkv_sb = [consts.tile([P, 2 * HD], DTB, tag=f"wkv{kc}", name=f"wkv{kc}") for kc in range(KC)]
        wo_sb = [consts.tile([P, D], DTB, tag=f"wo{c}", name=f"wo{c}") for c in range(2)]
        bq_sb = [consts.tile([P, 1], DT, tag=f"bq{mc}", name=f"bq{mc}") for mc in range(2)]
        bk_sb = consts.tile([HD, 1], DT, tag="bk")
        bv_sb = [consts.tile([P, 1], DT, tag=f"bv{c}", name=f"bv{c}") for c in range(2)]

        # ---- weight / bias / xT loads (round-robin over HWDGE queues) ----
        QS = [nc.sync, nc.scalar, nc.gpsimd]
        qi = 0

        def dq():
            nonlocal qi
            qi += 1
            return QS[qi % len(QS)]

        # kv weights + first x column-chunk first so the PE can start ASAP;
        # x is loaded in per-nb column chunks to pipeline DMA with proj.
        for kc in range(KC):
            dq().dma_start(wkv_sb[kc][:, 0:HD], wkd[kc * P:(kc + 1) * P, :])
            dq().dma_start(wkv_sb[kc][:, HD:2 * HD], wvd[kc * P:(kc + 1) * P, :])
        dq().dma_start(bk_sb, bkd[:, :])
        for dc in range(KC):
            QS[dc % len(QS)].dma_start(xT[dc][:, 0:512], xtd[dc * P:(dc + 1) * P, 0:512])
        for kc in range(KC):
            dq().dma_start(wq_sb[kc], wqd[kc * P:(kc + 1) * P, :])
        for c in range(2):
            dq().dma_start(bq_sb[c], bqd[c * P:(c + 1) * P, :])
            dq().dma_start(bv_sb[c], bvd[c * P:(c + 1) * P, :])
        for nb in range(1, 4):
            for dc in range(KC):
                QS[dc % len(QS)].dma_start(
                    xT[dc][:, nb * 512:(nb + 1) * 512],
                    xtd[dc * P:(dc + 1) * P, nb * 512:(nb + 1) * 512])
        for c in range(2):
            dq().dma_start(wo_sb[c], wod[c * P:(c + 1) * P, :])
        nc.vector.memset(vA[:, :, HD:HD + 1], 1.0)

        # ---- phase 1: projections (v transposed after q-proj so the PE
        # isn't gated on the vt copy) ----
        vts = []
        for nb in range(4):
            sl = slice(nb * 512, (nb + 1) * 512)
            ps2 = psOut.tile([P, 512], DT, tag="ps")
            for kc in range(KC):
                nc.tensor.matmul(
                    ps2, wkv_sb[kc], xT[kc][:, sl],
                    start=(kc == 0), stop=(kc == KC - 1))
            nc.scalar.activation(kT2[0:HD, sl], ps2[0:HD, :], AF.Identity, bias=bk_sb)
            nc.vector.tensor_copy(kT2[HD:2 * HD, sl], kT2[0:HD, sl])
            vt = vtmp.tile([HD, 512], DTB, tag=f"vtmp{nb}", name=f"vt{nb}")
            nc.vector.tensor_copy(vt, ps2[HD:2 * HD, :])
            vts.append(vt)
            for mc in range(2):
                ps = psOut.tile([P, 512], DT, tag="ps")
                for kc in range(KC):
                    nc.tensor.matmul(
                        ps, wq_sb[kc][:, mc * P:(mc + 1) * P], xT[kc][:, sl],
                        start=(kc == 0), stop=(kc == KC - 1))
                nc.scalar.activation(
                    qp[mc][:, sl], ps, AF.Identity, bias=bq_sb[mc], scale=0.125)

        def v_transposes(nb):
            for j in range(4):
                kt = nb * 4 + j
                vps = psOut.tile([P, HD], DTB, tag="ps", name="vps")
                nc.tensor.transpose(
                    vps, vts[nb][:, j * P:(j + 1) * P], ident[0:HD, 0:HD])
                nc.vector.tensor_copy(vA[:, kt, 0:HD], vps)

        # ---- phase 2: attention + interleaved output projection ----
        # v-transposes for block nb are emitted just before the attention
        # block that first needs them, so they don't head-of-line block the
        # PE on the vt copy.
        ydma = 0
        for qb in range(NQB):
            v_transposes(qb)
            qsl = slice(qb * 512, (qb + 1) * 512)
            nkt = 4 * (qb + 1)
            for mc in range(2):
                acc = [psAcc.tile([HD + 1, 512], DT, tag=f"acc{hh}",
                                  name=f"acc{hh}") for hh in range(2)]
                for kt in range(nkt):
                    ksl = slice(kt * P, (kt + 1) * P)
                    sps = psS.tile([P, 2, 512], DT, tag="sd")
                    nc.tensor.matmul(
                        sps[:, 0], kT2[0:HD, ksl], qp[mc][0:HD, qsl],
                        start=True, stop=True)
                    nc.tensor.matmul(
                        sps[:, 1], kT2[HD:2 * HD, ksl], qp[mc][HD:2 * HD, qsl],
                        start=True, stop=True)
                    et = etp.tile([P, 2, 512], DTB, tag="et")
                    nc.scalar.activation(et, sps, AF.Exp)
                    if kt >= 4 * qb:
                        di = kt - 4 * qb
                        nc.vector.tensor_mul(et, et, dmask[di])
                    for hh in range(2):
                        nc.tensor.matmul(
                            acc[hh], vA[:, kt, :], et[:, hh],
                            start=(kt == 0), stop=(kt == nkt - 1))
                for hh in range(2):
                    # Drain PSUM to SBUF immediately so the acc bank frees
                    # for the next (qb, mc) stream; normalize from SBUF.
                    den = recp.tile([1, 512], DT, tag="den")
                    nc.vector.tensor_copy(den, acc[hh][HD:HD + 1, :])
                    sba = accsb.tile([HD, 512], DT, tag="sba")
                    nc.vector.tensor_copy(sba, acc[hh][0:HD, :])
                    rec = recp.tile([1, 512], DT, tag="rec")
                    nc.vector.reciprocal_approx_fast(rec, den)
                    rbs = rbp.tile([HD, 512], DT, tag="rbs")
                    nc.gpsimd.partition_broadcast(rbs, rec)
                    r0 = hh * HD
                    nc.vector.tensor_mul(
                        oT[mc][r0:r0 + HD, qsl], sba, rbs)
                    nc.vector.tensor_scalar_add(
                        oT[mc][r0:r0 + HD, qsl], oT[mc][r0:r0 + HD, qsl],
                        bv_sb[mc][r0:r0 + HD, :])
            # output projection for this query block (all 4 heads ready)
            for st in range(qb * 4, qb * 4 + 4):
                for nb2 in range(2):
                    yps = psOut.tile([P, 512], DT, tag="ps")
                    for c in range(2):
                        nc.tensor.matmul(
                            yps, oT[c][:, st * P:(st + 1) * P],
                            wo_sb[c][:, nb2 * 512:(nb2 + 1) * 512],
                            start=(c == 0), stop=(c == 1))
                    yt = ysb.tile([P, 512], DT, tag="y")
                    nc.vector.tensor_copy(yt, yps)
                    QS[ydma % len(QS)].dma_start(
                        out_p[st * P:(st + 1) * P, nb2 * 512:(nb2 + 1) * 512], yt)
                    ydma += 1

    nc.compile()
    return nc


def kernel(x, mask, wq, bq, wk, bk, wv, bv, wo, bo):
    x = np.asarray(x, dtype=np.float32)
    wq = np.asarray(wq, dtype=np.float32)
    wk = np.asarray(wk, dtype=np.float32)
    wv = np.asarray(wv, dtype=np.float32)
    wo = np.asarray(wo, dtype=np.float32)
    bq = np.asarray(bq, dtype=np.float32)
    bk = np.asarray(bk, dtype=np.float32)
    bv = np.asarray(bv, dtype=np.float32)
    bo = np.asarray(bo, dtype=np.float32)

    wqb = wq.astype(BF16)
    wkb = wk.astype(BF16)
    wvb = wv.astype(BF16)
    wob = wo.astype(BF16)
    xtb = np.ascontiguousarray(x.transpose(0, 2, 1)).astype(BF16)  # [B, D, S]

    in_maps = []
    for c in range(N_CORES):
        b, g = c // 4, c % 4
        sq = slice(g * DG, (g + 1) * DG)
        sk = slice(g * HD, (g + 1) * HD)
        in_maps.append({
            "xtd": xtb[b],
            "wqd": np.ascontiguousarray(wqb[:, sq]),
            "wkd": np.ascontiguousarray(wkb[:, sk]),
            "wvd": np.ascontiguousarray(wvb[:, sk]),
            "wod": np.ascontiguousarray(wob[sq, :]),
            "bqd": np.ascontiguousarray((bq[sq] * 0.125).reshape(DG, 1)),
            "bkd": np.ascontiguousarray(bk[sk].reshape(HD, 1)),
            "bvd": np.ascontiguousarray(np.tile(bv[sk], GQ).reshape(DG, 1)),
        })

    results = _run(in_maps)

    out = np.empty((B, S, D), dtype=np.float32)
    for b in range(B):
        acc = results[b * 4 + 0]["out_p"].astype(np.float64)
        for g in range(1, 4):
            acc += results[b * 4 + g]["out_p"]
        out[b] = (acc + bo).astype(np.float32)
    return out


def _get_runner():
    """Build (once) a jitted shard_map callable executing the compiled
    kernel on 8 cores. Adapted from concourse.bass2jax.run_bass_via_pjrt,
    minus output-buffer donation so the callable is re-invokable for
    timing."""
    if "runner" in _CACHE:
        return _CACHE["runner"]
    import jax
    from jax.experimental.shard_map import shard_map
    from jax.sharding import Mesh, PartitionSpec
    from concourse import bass2jax
    from concourse.bass2jax import _bass_exec_p, install_neuronx_cc_hook

    install_neuronx_cc_hook()
    nc = build_nc()
    partition_name = (
        nc.partition_id_tensor.name if nc.partition_id_tensor else None
    )

    in_names, out_names, out_avals, zero_outs = [], [], [], []
    for alloc in nc.m.functions[0].allocations:
        if not isinstance(alloc, mybir.MemoryLocationSet):
            continue
        name = alloc.memorylocations[0].name
        if alloc.kind == "ExternalInput":
            if name != partition_name:
                in_names.append(name)
        elif alloc.kind == "ExternalOutput":
            out_names.append(name)
            shape = tuple(alloc.tensor_shape)
            dtype = mybir.dt.np(alloc.dtype)
            out_avals.append(jax.core.ShapedArray(shape, dtype))
            zero_outs.append(np.zeros(shape, dtype))
    n_params = len(in_names)
    all_names = in_names + out_names
    if partition_name is not None:
        all_names = all_names + [partition_name]

    def _body(*args):
        operands = list(args)
        if partition_name is not None:
            operands.append(bass2jax.partition_id_tensor())
        outs = _bass_exec_p.bind(
            *operands,
            out_avals=tuple(out_avals),
            in_names=tuple(all_names),
            out_names=tuple(out_names),
            lowering_input_output_aliases=(),
            sim_require_finite=True,
            sim_require_nnan=True,
            nc=nc,
        )
        return tuple(outs)

    devices = jax.devices()[:N_CORES]
    mesh = Mesh(np.asarray(devices), ("core",))
    n_all = n_params + len(out_names)
    sharded = jax.jit(
        shard_map(
            _body,
            mesh=mesh,
            in_specs=(PartitionSpec("core"),) * n_all,
            out_specs=(PartitionSpec("core"),) * len(out_names),
            check_rep=False,
        ),
        keep_unused=True,
    )
    runner = {
        "sharded": sharded,
        "in_names": in_names,
        "out_names": out_names,
        "out_avals": out_avals,
        "zero_outs": zero_outs,
        "mesh": mesh,
        "nc": nc,
    }
    _CACHE["runner"] = runner
    return runner


def _run(in_maps):
    r = _get_runner()
    concat_in = [
        np.concatenate([np.asarray(in_maps[c][n]) for c in range(N_CORES)], axis=0)
        for n in r["in_names"]
    ]
    concat_zeros = [
        np.zeros((N_CORES * z.shape[0], *z.shape[1:]), z.dtype)
        for z in r["zero_outs"]
    ]
    out_arrs = r["sharded"](*concat_in, *concat_zeros)
    _CACHE["last_args"] = (concat_in, concat_zeros)
    return [
        {
            n: np.asarray(out_arrs[i]).reshape(
                N_CORES, *r["out_avals"][i].shape
            )[c]
            for i, n in enumerate(r["out_names"])
        }
        for c in range(N_CORES)
    ]


def bench(iters=10):
    """Re-execute the last-run kernel with device-resident inputs and
    return per-call wall times (s). Outputs stay on device."""
    import time as _time
    import jax
    from jax.sharding import NamedSharding, PartitionSpec

    r = _CACHE["runner"]
    concat_in, concat_zeros = _CACHE["last_args"]
    sh = NamedSharding(r["mesh"], PartitionSpec("core"))
    dev_args = [jax.device_put(a, sh) for a in (*concat_in, *concat_zeros)]
    for a in dev_args:
        a.block_until_ready()
    times = []
    for _ in range(iters):
        t0 = _time.perf_counter()
        outs = r["sharded"](*dev_args)
        for o in outs:
            o.block_until_ready()
        times.append(_time.perf_counter() - t0)
    return times


def profile_exec_ns(outdir="/tmp/kernel_ntff"):
    """Capture an NTFF profile of one execution on all 8 cores and return
    (max_core_span_ns, per_core_span_ns). The span is the on-device NEFF
    execution time: last instruction end - first instruction start."""
    import ctypes
    import glob
    import json
    import shutil
    import jax
    from jax.sharding import NamedSharding, PartitionSpec

    r = _CACHE["runner"]
    concat_in, concat_zeros = _CACHE["last_args"]
    sh = NamedSharding(r["mesh"], PartitionSpec("core"))
    dev_args = [jax.device_put(a, sh) for a in (*concat_in, *concat_zeros)]
    for a in dev_args:
        a.block_until_ready()
    outs = r["sharded"](*dev_args)  # warm
    for o in outs:
        o.block_until_ready()

    lib = ctypes.CDLL("/opt/axon/libaxon_pjrt.so")
    if not hasattr(lib, "axon_start_nrt_profile"):
        return None, None
    lib.axon_start_nrt_profile.argtypes = [
        ctypes.POINTER(ctypes.c_int64), ctypes.c_size_t]
    lib.axon_start_nrt_profile.restype = ctypes.c_int64
    lib.axon_stop_nrt_profile.argtypes = [ctypes.c_char_p]
    lib.axon_stop_nrt_profile.restype = ctypes.c_int64

    shutil.rmtree(outdir, ignore_errors=True)
    os.makedirs(outdir, exist_ok=True)
    ids = (ctypes.c_int64 * N_CORES)(*range(N_CORES))
    rc = lib.axon_start_nrt_profile(ids, N_CORES)
    if rc != 0:
        return None, None
    outs = r["sharded"](*dev_args)
    for o in outs:
        o.block_until_ready()
    n = lib.axon_stop_nrt_profile(str(outdir).encode())
    if n <= 0:
        return None, None

    import gauge.profiler
    from concourse._compat import FishPath

    profile = gauge.profiler.Profile(
        profile_path=FishPath(outdir),
        kernel_dev_mode=True,
        profile_on_exit=False,
        bass_kernel=_CACHE["runner"]["nc"].m,
        offline_processing=True,
        fname="*_body*",
    )
    profile.convert_ntffs_to_json(tuple(range(N_CORES)))
    spans = {}
    for c in range(N_CORES):
        jp = os.path.join(outdir, f"ntff_{c}.json")
        if not os.path.exists(jp):
            continue
        d = json.load(open(jp))
        insts = d.get("instruction", [])
        if not insts:
            continue
        t0 = min(i["timestamp"] for i in insts)
        t1 = max(i["timestamp"] + i["duration"] for i in insts)
        spans[c] = t1 - t0
    if not spans:
        return None, None
    return max(spans.values()), spans
